# revision 1
# baseline (speedup 1.0000x reference)
"""Multi-head graph-attention (GAT) kernel for Trainium2, 8 NeuronCores.

Reference computation (per head):
    h_prime = h @ w[head]                       # [8192, 64]
    s = h_prime @ a_src[head],  d = h_prime @ a_dst[head]
    attn = softmax_j(leaky_relu(s_i + d_j, 0.2))
    out  = attn @ h_prime + bias                # -> [8192, 4*64]

Key identity: with exp monotone,
    exp(lrelu(s_i + d_j)) = e^{s_i} e^{d_j}           if s_i + d_j >= 0
                          = e^{0.2 s_i} e^{0.2 d_j}   otherwise
so with the 0/1 mask M[j,i] = [s_i + d_j >= 0] and row-scaled matrices
Hv = h' * e^d, Hq = h' * e^{0.2 d} (and vectors v = e^d, q = e^{0.2 d}):
    num[:,i]  = e^{s_i} (Hv^T M)[:,i] + e^{0.2 s_i} (Sq - (Hq^T M)[:,i])
    den[i]    = e^{s_i} (v^T M)[i]   + e^{0.2 s_i} (sum(q) - (q^T M)[i])
Only the mask is an O(n^2) elementwise op (one DVE tensor_scalar per tile);
the O(n^2) contraction runs on the PE as fp16 matmuls over the mask
(fp16's 10-bit mantissa keeps the result ~3e-5 relative; the fp32
corrections Sq / sum(q) are applied exactly in the epilogue):
  - mm1: stationary [Hv | Hq] fp16 ([128j, 128]) x mask -> AC psum
  - mm2: stationary [v | -q]  fp16 ([128j, 2])   x mask -> den psum
Two i-chunks are processed per stationary load: the second matmul of each
pair sets InstMatmult.ldweights=False to reuse the already-loaded weights
(LDWEIGHTS is serial with MATMUL on TRN2, ~95ns per 128-col fp16 load).

Sharding: 8 cores = 4 heads x 2 row-halves (head parallel + bs row shard).
Each core gets full h (2 MB) plus its row block; no collectives.
"""

import numpy as np

import concourse.bass as bass
import concourse.tile as tile
from concourse import bacc, mybir
from concourse.bass_utils import run_bass_kernel_spmd
from concourse.masks import make_identity

F32 = mybir.dt.float32
BF16 = mybir.dt.bfloat16
FP16 = mybir.dt.float16
AF = mybir.ActivationFunctionType
OP = mybir.AluOpType

BS = 8192          # nodes
F = 64             # f_in == f_out
NH = 4             # heads
HALF = BS // 2     # rows per core (row-half)
NT_J = BS // 128   # 64 j tiles
NT_I = HALF // 128 # 32 i tiles
NCH = HALF // 512  # 8 i chunks of 512
ALPHA = 0.2


def _build_kernel_module():
    nc = bacc.Bacc("TRN2", target_bir_lowering=False, debug=False)

    hfull_d = nc.dram_tensor("hfull", [BS, F], F32, kind="ExternalInput")
    hblk_d = nc.dram_tensor("hblk", [HALF, F], F32, kind="ExternalInput")
    w_d = nc.dram_tensor("w", [F, F], F32, kind="ExternalInput")
    aa_d = nc.dram_tensor("aa", [F, 2], F32, kind="ExternalInput")
    bias_d = nc.dram_tensor("bias", [1, F], F32, kind="ExternalInput")
    out_d = nc.dram_tensor("out", [HALF, F], F32, kind="ExternalOutput")

    with tile.TileContext(nc) as tc:
        with (
            tc.tile_pool(name="const", bufs=1) as cpool,
            tc.tile_pool(name="work", bufs=3) as wpool,
            tc.tile_pool(name="psum", bufs=2, space="PSUM") as ppool,
        ):
            # ---------------- constants ----------------
            identity = cpool.tile([128, 128], F32)
            make_identity(nc, identity[:])
            ones = cpool.tile([128, 512], F32)
            nc.gpsimd.memset(ones[:], 1.0)

            # ---------------- tiny weight prep ----------------
            w_sb = cpool.tile([F, F], F32)
            nc.sync.dma_start(w_sb[:], w_d.ap())
            aa_sb = cpool.tile([F, 2], F32)
            nc.sync.dma_start(aa_sb[:], aa_d.ap())
            bias_sb = cpool.tile([1, F], F32)
            nc.sync.dma_start(bias_sb[:], bias_d.ap())

            wT_ps = ppool.tile([F, F], F32, tag="mix")
            nc.tensor.transpose(wT_ps[:], w_sb[:], identity[0:F, 0:F])
            wT_sb = cpool.tile([F, F], F32)
            nc.scalar.copy(wT_sb[:], wT_ps[:])

            # ws = w @ [a_src | a_dst]  -> [64, 2]
            ws_ps = ppool.tile([F, 2], F32, tag="mix")
            nc.tensor.matmul(ws_ps[:], wT_sb[:], aa_sb[:])
            ws_sb = cpool.tile([F, 2], F32)
            nc.scalar.copy(ws_sb[:], ws_ps[:])

            # w_aug = [w | w@a_dst]  (h @ w_aug gives h_prime and d at once)
            w_aug = cpool.tile([F, F + 1], F32)
            nc.scalar.copy(w_aug[:, 0:F], w_sb[:])
            nc.scalar.copy(w_aug[:, F : F + 1], ws_sb[:, 1:2])

            # bias broadcast to all partitions
            biasb_ps = ppool.tile([128, F], F32, tag="mix")
            nc.tensor.matmul(biasb_ps[:], ones[0:1, 0:128], bias_sb[:])
            bias_rep = cpool.tile([128, F], F32)
            nc.scalar.copy(bias_rep[:], biasb_ps[:])

            # ---------------- row-block path first (feeds masks early) -------
            hbT = cpool.tile([F, HALF], F32)   # (row block of h)^T
            hb_view = hblk_d.ap().rearrange("(a p) f -> p a f", p=128)
            for blk in range(NT_I // 8):
                ldb = wpool.tile([128, 8 * F], F32, tag="hloadb", bufs=2)
                nc.sync.dma_start(
                    ldb[:], hb_view[:, blk * 8 : (blk + 1) * 8, :]
                )
                for k in range(8):
                    it = blk * 8 + k
                    tr = ppool.tile([F, 128], F32, tag="mix")
                    nc.tensor.transpose(
                        tr[:], ldb[:, k * F : (k + 1) * F], identity[:]
                    )
                    if k % 2 == 0:
                        nc.scalar.copy(hbT[:, it * 128 : (it + 1) * 128], tr[:])
                    else:
                        nc.vector.tensor_copy(
                            hbT[:, it * 128 : (it + 1) * 128], tr[:]
                        )

            s_row = cpool.tile([1, HALF], F32)
            for ch in range(NCH):
                sr_ps = ppool.tile([1, 512], F32, tag="mix")
                nc.tensor.matmul(
                    sr_ps[:], ws_sb[:, 0:1], hbT[:, ch * 512 : (ch + 1) * 512]
                )
                nc.scalar.copy(s_row[:, ch * 512 : (ch + 1) * 512], sr_ps[:])

            s_col = cpool.tile([128, NT_I], F32)
            for it in range(NT_I):
                sc_ps = ppool.tile([128, 1], F32, tag="mix")
                nc.tensor.matmul(
                    sc_ps[:], hbT[:, it * 128 : (it + 1) * 128], ws_sb[:, 0:1]
                )
                nc.scalar.copy(s_col[:, it : it + 1], sc_ps[:])

            u_col = cpool.tile([128, NT_I], F32)   # e^s
            nc.scalar.activation(u_col[:], s_col[:], AF.Exp)
            p_col = cpool.tile([128, NT_I], F32)   # e^{0.2 s}
            nc.scalar.activation(p_col[:], s_col[:], AF.Exp, scale=ALPHA)
            np_col = cpool.tile([128, NT_I], F32)  # -e^{0.2 s}
            nc.vector.tensor_scalar_mul(np_col[:], p_col[:], -1.0)

            # s replicated across partitions, bf16 (mask input)
            s_rep_b = cpool.tile([128, HALF], BF16)
            for ch in range(NCH):
                sb_ps = ppool.tile([128, 512], F32, tag="mix")
                nc.tensor.matmul(
                    sb_ps[:],
                    ones[0:1, 0:128],
                    s_row[:, ch * 512 : (ch + 1) * 512],
                )
                nc.scalar.copy(s_rep_b[:, ch * 512 : (ch + 1) * 512], sb_ps[:])

            # ---------------- h^T and h_prime (+d) for all j ----------------
            hT = cpool.tile([F, BS], F32)      # h^T, K-major for PE
            hpr = cpool.tile([128, NT_J * F], F32)     # h_prime, fp32
            hpr3 = hpr[:].rearrange("p (t c) -> p t c", c=F)
            d_col = cpool.tile([128, NT_J], F32)
            hf_view = hfull_d.ap().rearrange("(a p) f -> p a f", p=128)
            for blk in range(NT_J // 8):
                ldb = wpool.tile([128, 8 * F], F32, tag="hloadb", bufs=2)
                nc.sync.dma_start(
                    ldb[:], hf_view[:, blk * 8 : (blk + 1) * 8, :]
                )
                for k in range(8):
                    jt = blk * 8 + k
                    tr = ppool.tile([F, 128], F32, tag="mix")
                    nc.tensor.transpose(
                        tr[:], ldb[:, k * F : (k + 1) * F], identity[:]
                    )
                    if k % 2 == 0:
                        nc.scalar.copy(hT[:, jt * 128 : (jt + 1) * 128], tr[:])
                    else:
                        nc.vector.tensor_copy(
                            hT[:, jt * 128 : (jt + 1) * 128], tr[:]
                        )
                    hp_ps = ppool.tile([128, F + 1], F32, tag="mix")
                    nc.tensor.matmul(
                        hp_ps[:], hT[:, jt * 128 : (jt + 1) * 128], w_aug[:]
                    )
                    if k % 2 == 0:
                        nc.vector.tensor_copy(hpr3[:, jt, :], hp_ps[:, 0:F])
                    else:
                        nc.scalar.copy(hpr3[:, jt, :], hp_ps[:, 0:F])
                    nc.vector.tensor_copy(
                        d_col[:, jt : jt + 1], hp_ps[:, F : F + 1]
                    )

            # per-16-tile groups: exps of d, VQ and HH builds (lets the main
            # loop start as soon as the first tiles are ready)
            GRP = 16
            v_col = cpool.tile([128, NT_J], F32)
            q_col = cpool.tile([128, NT_J], F32)
            nq_col = cpool.tile([128, NT_J], F32)
            negd_col = cpool.tile([128, NT_J], F32)
            VQ = cpool.tile([128, NT_J * 2], FP16)
            VQ3 = VQ[:].rearrange("p (t c) -> p t c", c=2)
            HH = cpool.tile([128, NT_J * 128], FP16)
            HH3 = HH[:].rearrange("p (t c) -> p t c", c=128)
            sq_ps = ppool.tile([1, F], F32, tag="sq", bufs=1)
            for g in range(NT_J // GRP):
                gs = slice(g * GRP, (g + 1) * GRP)
                nc.scalar.activation(v_col[:, gs], d_col[:, gs], AF.Exp)
                nc.scalar.activation(q_col[:, gs], d_col[:, gs], AF.Exp, scale=ALPHA)
                nc.vector.tensor_scalar_mul(nq_col[:, gs], q_col[:, gs], -1.0)
                nc.vector.tensor_scalar_mul(negd_col[:, gs], d_col[:, gs], -1.0)
                nc.vector.tensor_copy(VQ3[:, gs, 0], v_col[:, gs])
                nc.vector.tensor_copy(VQ3[:, gs, 1], nq_col[:, gs])
                for jt in range(g * GRP, (g + 1) * GRP):
                    stage = wpool.tile([128, 128], F32, tag="stage", bufs=3)
                    nc.vector.tensor_scalar_mul(
                        stage[:, 0:F], hpr3[:, jt, :], v_col[:, jt : jt + 1]
                    )
                    nc.vector.tensor_scalar_mul(
                        stage[:, F:128], hpr3[:, jt, :], q_col[:, jt : jt + 1]
                    )
                    nc.scalar.copy(HH3[:, jt, :], stage[:])
                    # Sq accumulation: ones^T @ Hq_f32
                    nc.tensor.matmul(
                        sq_ps[:],
                        ones[:, 0:1],
                        stage[:, F:128],
                        start=(jt == 0),
                        stop=(jt == NT_J - 1),
                    )
            sq_sb = cpool.tile([1, F], F32)
            nc.scalar.copy(sq_sb[:], sq_ps[:])
            # Sq broadcast to all partitions (fp32 exact correction)
            sqb_ps = ppool.tile([128, F], F32, tag="mix")
            nc.tensor.matmul(sqb_ps[:], ones[0:1, 0:128], sq_sb[:])
            Sq_rep = cpool.tile([128, F], F32)
            nc.scalar.copy(Sq_rep[:], sqb_ps[:])

            # Sq_tot = sum_j q_j, then pSqt[:, it] = p * Sq_tot (epilogue bias)
            qs_ps = ppool.tile([NT_J, 1], F32, tag="sq", bufs=1)
            nc.tensor.matmul(qs_ps[:], q_col[:], ones[:, 0:1])
            qs_sb = cpool.tile([NT_J, 1], F32)
            nc.scalar.copy(qs_sb[:], qs_ps[:])
            sqt_ps = ppool.tile([1, 1], F32, tag="sq", bufs=1)
            nc.tensor.matmul(sqt_ps[:], qs_sb[:], ones[0:NT_J, 0:1])
            sqt_sb = cpool.tile([1, 1], F32)
            nc.scalar.copy(sqt_sb[:], sqt_ps[:])
            sqtb_ps = ppool.tile([128, 1], F32, tag="sq", bufs=1)
            nc.tensor.matmul(sqtb_ps[:], ones[0:1, 0:128], sqt_sb[:])
            Sqt_col = cpool.tile([128, 1], F32)
            nc.scalar.copy(Sqt_col[:], sqtb_ps[:])
            pSqt = cpool.tile([128, NT_I], F32)
            nc.vector.tensor_scalar_mul(pSqt[:], p_col[:], Sqt_col[:])

            # ---------------- main flash loop (software-pipelined) -----------
            def epilogue(ch, AC_sb, den_sb, te=None):
                te = te or nc.gpsimd
                for sub in range(4):
                    it = ch * 4 + sub
                    ACt_ps = ppool.tile([128, 128], F32, tag="mix")
                    nc.tensor.transpose(
                        ACt_ps[:],
                        AC_sb[:, sub * 128 : (sub + 1) * 128],
                        identity[:],
                    )
                    dent_ps = ppool.tile([128, 2], F32, tag="mix")
                    nc.tensor.transpose(
                        dent_ps[:],
                        den_sb[:, sub * 128 : (sub + 1) * 128],
                        identity[0:2, 0:2],
                    )
                    # numerator = u*A_T + p*Sq - p*C_T
                    t1 = wpool.tile([128, F], F32, tag="t1", bufs=2)
                    nc.scalar.activation(
                        t1[:], ACt_ps[:, 0:F], AF.Identity,
                        scale=u_col[:, it : it + 1],
                    )
                    cT = wpool.tile([128, F], F32, tag="cT", bufs=2)
                    nc.scalar.activation(
                        cT[:], ACt_ps[:, F:128], AF.Identity,
                        scale=np_col[:, it : it + 1],
                    )
                    pSq = wpool.tile([128, F], F32, tag="pSq", bufs=2)
                    te.tensor_scalar_mul(
                        pSq[:], Sq_rep[:], p_col[:, it : it + 1]
                    )
                    n1 = wpool.tile([128, F], F32, tag="n1", bufs=2)
                    te.tensor_add(n1[:], t1[:], cT[:])
                    num = wpool.tile([128, F], F32, tag="num", bufs=2)
                    te.tensor_add(num[:], n1[:], pSq[:])
                    # denominator = u*(vM) + p*Sqt - p*(qM)
                    y1 = wpool.tile([128, 1], F32, tag="y1", bufs=2)
                    nc.scalar.activation(
                        y1[:], dent_ps[:, 0:1], AF.Identity,
                        scale=u_col[:, it : it + 1],
                    )
                    y2 = wpool.tile([128, 1], F32, tag="y2", bufs=2)
                    nc.scalar.activation(
                        y2[:], dent_ps[:, 1:2], AF.Identity,
                        scale=p_col[:, it : it + 1],
                        bias=pSqt[:, it : it + 1],
                    )
                    den = wpool.tile([128, 1], F32, tag="den", bufs=2)
                    te.tensor_add(den[:], y1[:], y2[:])
                    rec = wpool.tile([128, 1], F32, tag="rec", bufs=2)
                    nc.vector.reciprocal(rec[:], den[:])
                    o_t = wpool.tile([128, F], F32, tag="ot", bufs=2)
                    nc.scalar.activation(
                        o_t[:], num[:], AF.Identity, scale=rec[:]
                    )
                    o_f = wpool.tile([128, F], F32, tag="of", bufs=2)
                    te.tensor_add(o_f[:], o_t[:], bias_rep[:])
                    nc.sync.dma_start(
                        out_d.ap()[it * 128 : (it + 1) * 128, :], o_f[:]
                    )

            pending = []
            for grp in range(NCH // 4):
                while pending:
                    epilogue(*pending.pop(0))
                ACs = [
                    ppool.tile([128, 512], F32, tag="acc", bufs=4, name=f"AC{c}")
                    for c in range(4)
                ]
                den4 = ppool.tile([98, 512], F32, tag="dacc", bufs=1)
                chs = [4 * grp + c for c in range(4)]
                for jt in range(NT_J):
                    ms = []
                    for half2 in range(2):
                        mw = wpool.tile([128, 1024], FP16, tag="mask", bufs=4,
                                        name=f"mw{half2}")
                        c0 = chs[2 * half2]
                        nc.vector.tensor_scalar(
                            mw[:], s_rep_b[:, c0 * 512 : (c0 + 2) * 512],
                            negd_col[:, jt : jt + 1], None, OP.is_ge,
                        )
                        ms.append(mw[:, 0:512])
                        ms.append(mw[:, 512:1024])
                    st, sp = (jt == 0), (jt == NT_J - 1)
                    for c in range(4):
                        i = nc.tensor.matmul(
                            ACs[c][:], HH3[:, jt, :], ms[c][:], start=st, stop=sp
                        )
                        if c > 0:
                            i.ins.ldweights = False
                    for c in range(4):
                        nc.tensor.matmul(
                            den4[32 * c : 32 * c + 2, :], VQ3[:, jt, :], ms[c][:],
                            start=st, stop=sp, tile_position=(0, 32 * c),
                        )
                for c in range(4):
                    AC_sb = wpool.tile([128, 512], F32, tag="ACsb", bufs=5,
                                       name=f"ACsb{c}")
                    nc.scalar.copy(AC_sb[:], ACs[c][:])
                    den_sb = wpool.tile([2, 512], F32, tag="densb", bufs=5,
                                        name=f"densb{c}")
                    nc.scalar.copy(den_sb[:], den4[32 * c : 32 * c + 2, :])
                    pending.append((chs[c], AC_sb, den_sb[:]))
            for k, args in enumerate(pending):
                epilogue(*args, te=(nc.vector if k % 2 == 0 else nc.gpsimd))

    nc.compile()
    return nc


_NC_CACHE = None


def _get_nc():
    global _NC_CACHE
    if _NC_CACHE is None:
        _NC_CACHE = _build_kernel_module()
    return _NC_CACHE


def _make_in_maps(h, w, a_src, a_dst, bias):
    h = np.ascontiguousarray(np.asarray(h, dtype=np.float32))
    w = np.asarray(w, dtype=np.float32)
    a_src = np.asarray(a_src, dtype=np.float32)
    a_dst = np.asarray(a_dst, dtype=np.float32)
    bias = np.asarray(bias, dtype=np.float32).reshape(1, F)
    in_maps = []
    for c in range(8):
        head, half = c // 2, c % 2
        aa = np.ascontiguousarray(
            np.concatenate([a_src[head], a_dst[head]], axis=1)
        )
        in_maps.append(
            {
                "hfull": h,
                "hblk": np.ascontiguousarray(h[half * HALF : (half + 1) * HALF]),
                "w": np.ascontiguousarray(w[head]),
                "aa": aa,
                "bias": bias,
            }
        )
    return in_maps


def _run(h, w, a_src, a_dst, bias, trace=False, **trace_kwargs):
    nc = _get_nc()
    in_maps = _make_in_maps(h, w, a_src, a_dst, bias)
    res = run_bass_kernel_spmd(
        nc, in_maps, core_ids=list(range(8)), trace=trace, **trace_kwargs
    )
    out = np.zeros((BS, NH * F), dtype=np.float32)
    for c in range(8):
        head, half = c // 2, c % 2
        out[half * HALF : (half + 1) * HALF, head * F : (head + 1) * F] = res.results[
            c
        ]["out"]
    return out, res


def kernel(h, w, a_src, a_dst, bias):
    out, _ = _run(h, w, a_src, a_dst, bias, trace=False)
    return out

